# revision 26
# baseline (speedup 1.0000x reference)
"""Trainium2 Bass kernel for nn_Attention_49168785605257.

Causal multi-head self-attention: B=2, N=4096, DIM=512, H=8, DH=64.
Reference applies dim_head**-0.5 scaling TWICE (folded here into Wq as 1/64).

Sharding: one head per NeuronCore (8 cores). Each core computes its head's
attention for BOTH batches (packed into partition halves 0-63 / 64-127) and
its partial output projection o_h = attnU_h @ Wo[64h:64h+64, :] UNNORMALIZED,
plus the per-token softmax denominators den_h.  The host computes
sum_h(o_h / den_h) + bias.

Device-side formulation (per core):
  - All tensors carried transposed ([feature, token]); host pre-transposes x.
  - Flash-attention in S^T orientation, pipelined one j-block ahead: the
    score matmuls for step s+1 are emitted before the A@V matmuls of step s
    so the scalar engine (exp, the bottleneck) never waits at chunk
    boundaries.
  - exp on ScalarE (PSUM->SBUF [128,1024] per j-block covering both
    batches); causal masking via 0/1-mask multiply on the exp output of the
    diagonal j-blocks; A@V accumulated in PSUM with v augmented by a
    ones-column so row 64 collects the denominators.
  - Normalization (1/den) deferred to the host: device DMAs den (fp32) and
    the unnormalized projected partial (fp16) per chunk.
  - Projection / output-projection work is spread one piece per j-block so
    the PE never bursts long enough to starve the scalar engine.
"""

import os
import sys
from contextlib import ExitStack

import numpy as np

for _p in ("/opt/trn_rl_repo", "/root/.axon_site/_ro/trn_rl_repo"):
    if _p not in sys.path and os.path.isdir(_p):
        sys.path.append(_p)

import ml_dtypes  # noqa: E402

B, N, DIM, H, DH = 2, 4096, 512, 8, 64
N_CORES = 8
CH = 512            # i-chunk width (tokens)
JB = 128            # j-block width (tokens)


def build_attention_kernel(nc, NB: int):
    """Emit the per-core program. NB = tokens per batch (4096 full size)."""
    import concourse.mybir as mybir
    import concourse.tile as tile

    bf16 = mybir.dt.bfloat16
    f16 = mybir.dt.float16
    f32 = mybir.dt.float32
    mult = mybir.AluOpType.mult
    Exp = mybir.ActivationFunctionType.Exp

    NCH = NB // CH          # i-chunks per batch
    JTB = NB // JB          # j-blocks per batch

    xT_d = nc.dram_tensor("xT", [DIM, 2 * NB], bf16, kind="ExternalInput").ap()
    wq_d = nc.dram_tensor("wq", [128, 4 * DH], bf16, kind="ExternalInput").ap()
    wk_d = nc.dram_tensor("wk", [128, 4 * DH], bf16, kind="ExternalInput").ap()
    wv_d = nc.dram_tensor("wv", [128, 4 * DH], bf16, kind="ExternalInput").ap()
    wo_d = nc.dram_tensor("wo", [DH, DIM], bf16, kind="ExternalInput").ap()
    mask_d = nc.dram_tensor("masks", [128, 4096], bf16, kind="ExternalInput").ap()
    idup_d = nc.dram_tensor("identup", [128, DH], bf16, kind="ExternalInput").ap()
    oT_d = nc.dram_tensor("oT", [DIM, 2 * NB], f16, kind="ExternalOutput").ap()
    den_d = nc.dram_tensor("den", [1, 2 * NB], f32, kind="ExternalOutput").ap()

    with tile.TileContext(nc) as tc, ExitStack() as ctx:
        const = ctx.enter_context(tc.tile_pool(name="const", bufs=1))
        xpool = ctx.enter_context(tc.tile_pool(name="xp", bufs=16))
        big = ctx.enter_context(tc.tile_pool(name="big", bufs=1))
        ptp = ctx.enter_context(tc.tile_pool(name="ptp", bufs=6))
        rp = ctx.enter_context(tc.tile_pool(name="rp", bufs=2))
        op_sb_pool = ctx.enter_context(tc.tile_pool(name="osb", bufs=4))
        wup = ctx.enter_context(tc.tile_pool(name="wup", bufs=1))
        ps_pool = ctx.enter_context(tc.tile_pool(name="ps", bufs=2, space="PSUM"))
        av_pool = ctx.enter_context(tc.tile_pool(name="av", bufs=1, space="PSUM"))
        pv_pool = ctx.enter_context(tc.tile_pool(name="pv", bufs=2, space="PSUM"))

        # ---- weights first so chunk-0 projections can start ASAP ----
        wq_sb = const.tile([128, 4 * DH], bf16, tag="wq")
        wk_sb = const.tile([128, 4 * DH], bf16, tag="wk")
        wv_sb = const.tile([128, 4 * DH], bf16, tag="wv")
        nc.sync.dma_start(wq_sb[:], wq_d[:, :])
        nc.sync.dma_start(wk_sb[:], wk_d[:, :])
        nc.sync.dma_start(wv_sb[:], wv_d[:, :])
        wo_sb = const.tile([DH, DIM], bf16, tag="wo")
        mask_sb = const.tile([128, 4096], bf16, tag="mask")
        idup_sb = const.tile([128, DH], bf16, tag="idup")

        # ---- persistent activations (partition halves: rows 0-63 batch0, 64-127 batch1) ----
        qT = big.tile([128, NB], bf16, tag="qT")
        kT = big.tile([128, NB], bf16, tag="kT")
        vT = big.tile([128, NB], bf16, tag="vT")
        vaug = [big.tile([128, 65 * JTB], bf16, tag=f"vaug{b}", name=f"vaug{b}")
                for b in range(2)]

        xts_pend = {}

        def emit_xt(c, split=False):
            """Issue the x-chunk DMAs for chunk c on the gpsimd queue (first
            chunk: split across sync+gpsimd so the parts land sooner)."""
            xts = []
            for d in range(4):
                xt = xpool.tile([128, 1024], bf16, tag="xt", name=f"xt{c}_{d}")
                eng = nc.sync if (split and d % 2 == 0) else nc.gpsimd
                eng.dma_start(xt[:], xT_d[128 * d:128 * (d + 1), 1024 * c:1024 * (c + 1)])
                xts.append(xt)
            xts_pend[c] = xts

        def emit_proj_part(c, pi):
            """One of the q/k/v projections for chunk c (0=q, 1=k, 2=v)."""
            i0 = CH * c
            xts = xts_pend[c]
            w_sb, dst = ((wq_sb, qT), (wk_sb, kT), (wv_sb, vT))[pi]
            ps = pv_pool.tile([128, CH], f32, tag="pv")
            for d in range(4):
                nc.tensor.matmul(ps[0:64, :], w_sb[:, d * DH:(d + 1) * DH], xts[d][:, 0:512],
                                 start=(d == 0), stop=(d == 3), tile_position=(0, 0),
                                 skip_group_check=True)
                nc.tensor.matmul(ps[64:128, :], w_sb[:, d * DH:(d + 1) * DH], xts[d][:, 512:1024],
                                 start=(d == 0), stop=(d == 3), tile_position=(0, 64),
                                 skip_group_check=True)
            nc.vector.tensor_copy(dst[:, i0:i0 + CH], ps[:, :])
            if pi == 2:
                del xts_pend[c]

        def emit_proj_qk(c):
            """q and k projections interleaved per x-part, so each arriving
            x DMA part is consumed immediately (startup critical path)."""
            i0 = CH * c
            xts = xts_pend[c]
            psq = pv_pool.tile([128, CH], f32, tag="pv", name="psq")
            psk = pv_pool.tile([128, CH], f32, tag="pv", name="psk")
            for d in range(4):
                for w_sb, ps in ((wq_sb, psq), (wk_sb, psk)):
                    nc.tensor.matmul(ps[0:64, :], w_sb[:, d * DH:(d + 1) * DH], xts[d][:, 0:512],
                                     start=(d == 0), stop=(d == 3), tile_position=(0, 0),
                                     skip_group_check=True)
                    nc.tensor.matmul(ps[64:128, :], w_sb[:, d * DH:(d + 1) * DH], xts[d][:, 512:1024],
                                     start=(d == 0), stop=(d == 3), tile_position=(0, 64),
                                     skip_group_check=True)
            nc.vector.tensor_copy(qT[:, i0:i0 + CH], psq[:, :])
            nc.vector.tensor_copy(kT[:, i0:i0 + CH], psk[:, :])

        def emit_transpose_tt(tt):
            """One V^T -> V (vaug) block via PE transposes."""
            pst0 = pv_pool.tile([128, 64], bf16, tag="pv", name="pst0")
            pst1 = pv_pool.tile([128, 64], bf16, tag="pv", name="pst1")
            nc.tensor.matmul(pst0[:], vT[0:64, JB * tt:JB * (tt + 1)], idup_sb[0:64, :],
                             is_transpose=True, tile_position=(0, 0), skip_group_check=True)
            nc.tensor.matmul(pst1[:], vT[64:128, JB * tt:JB * (tt + 1)], idup_sb[64:128, :],
                             is_transpose=True, tile_position=(64, 0), skip_group_check=True)
            nc.vector.tensor_copy(vaug[0][:, 65 * tt:65 * tt + 64], pst0[:])
            nc.vector.tensor_copy(vaug[1][:, 65 * tt:65 * tt + 64], pst1[:])

        def emit_transposes(c):
            """V^T -> V (vaug) for chunk c via PE transposes."""
            for tt in range(4 * c, 4 * c + 4):
                emit_transpose_tt(tt)

        def emit_epilogue_a(c, pso):
            """Evacuate pso, ship den, cast the unnormalized output to bf16."""
            outT_un = rp.tile([65, 1024], f32, tag="outT_un")
            nc.vector.tensor_copy(outT_un[:], pso[0:65, 0:1024])
            nc.sync.dma_start(den_d[0:1, 1024 * c:1024 * (c + 1)], outT_un[64:65, :])
            outTn = rp.tile([64, 1024], bf16, tag="outTn")
            nc.vector.tensor_copy(outTn[:], outT_un[0:64, :])
            return outTn

        def emit_outproj_unit(c, outTn, unit, tail=False):
            """One (dblk, batch) slice of the output projection; stream fp16."""
            dblk, b = unit // 2, unit % 2
            opp = pv_pool.tile([128, 512], f32, tag="pv", name=f"opp{b}")
            nc.tensor.matmul(opp[:], wo_sb[:, 128 * dblk:128 * (dblk + 1)],
                             outTn[:, 512 * b:512 * b + 512],
                             skip_group_check=True)
            o_sb = op_sb_pool.tile([128, 512], f16, tag="o")
            if tail and unit % 2 == 0:
                nc.scalar.copy(o_sb[:], opp[:])
            else:
                nc.vector.tensor_copy(o_sb[:], opp[:])
            nc.sync.dma_start(
                oT_d[128 * dblk:128 * (dblk + 1), 1024 * c + 512 * b:1024 * c + 512 * b + 512],
                o_sb[:])

        # ---- startup ----
        # PE warm-up overlapping the xt0 DMA wait: ~4.5us of dummy matmuls
        # covering a full HAM activity window so the clock gate opens before
        # the real projections start.
        wu_sb = wup.tile([128, 256], bf16, tag="wu")
        nc.vector.memset(wu_sb[:], 0.0)
        wu_ps = pv_pool.tile([128, 256], f32, tag="pv", name="wu")
        for i in range(20):
            nc.tensor.matmul(wu_ps[:], wu_sb[:, 128:256], wu_sb[:, 0:256],
                             start=(i == 0), stop=(i == 19), skip_group_check=True)
        emit_xt(0, split=True)
        emit_xt(1)
        nc.gpsimd.dma_start(mask_sb[:], mask_d[:, :])
        nc.sync.dma_start(idup_sb[:], idup_d[:, :])
        nc.vector.memset(vaug[0][:], 1.0)
        nc.vector.memset(vaug[1][:], 1.0)
        emit_proj_qk(0)
        nc.sync.dma_start(wo_sb[:], wo_d[:, :])

        def scores_exp(c, jb):
            """Score matmuls + exp (+ diag mask) for (c, jb). Returns AV args."""
            i0 = CH * c
            t = jb - 4 * c
            off = 128 * t if t > 0 else 0
            pss = ps_pool.tile([128, 1024], f32, tag="s")
            nc.tensor.matmul(pss[:, off:512], kT[0:64, JB * jb:JB * (jb + 1)],
                             qT[0:64, i0 + off:i0 + CH],
                             start=True, stop=True, tile_position=(0, 0), skip_group_check=True)
            nc.tensor.matmul(pss[:, 512 + off:1024], kT[64:128, JB * jb:JB * (jb + 1)],
                             qT[64:128, i0 + off:i0 + CH],
                             start=True, stop=True, tile_position=(64, 0), skip_group_check=True)
            pt = ptp.tile([128, 1024], bf16, tag="pt")
            if off:
                sub = lambda ap: ap.rearrange("p (h w) -> p h w", h=2)[:, :, off:]
                nc.scalar.activation(sub(pt[:]), sub(pss[:]), Exp)
                nc.vector.tensor_tensor(
                    sub(pt[:]), sub(pt[:]),
                    sub(mask_sb[:, 1024 * t:1024 * (t + 1)]), mult)
            else:
                nc.scalar.activation(pt[:], pss[:], Exp)
                if t == 0:
                    nc.vector.tensor_tensor(pt[:], pt[:], mask_sb[:, 0:1024], mult)
            return (c, jb, pt, off)

        def emit_av(prev, pso):
            c, jb, pt, off = prev
            njb = 4 * (c + 1)
            nc.tensor.matmul(pso[:, off:512], vaug[0][:, 65 * jb:65 * jb + 65], pt[:, off:512],
                             start=(jb == 0), stop=(jb == njb - 1), skip_group_check=True)
            nc.tensor.matmul(pso[:, 512 + off:1024], vaug[1][:, 65 * jb:65 * jb + 65],
                             pt[:, 512 + off:1024],
                             start=(jb == 0), stop=(jb == njb - 1), skip_group_check=True)

        # ---- micro-task queue for side work (proj d-tiles / transposes /
        # output-projection units), drained at a paced rate per step so the
        # PE insert between two exps never exceeds the exp window ----
        from collections import deque
        side_q = deque()
        proj_ps = {}

        def task_proj_d(c, pi, d):
            w_sb, dst = ((wq_sb, qT), (wk_sb, kT), (wv_sb, vT))[pi]
            xts = xts_pend[c]
            if d == 0:
                proj_ps[(c, pi)] = pv_pool.tile([128, CH], f32, tag="pv",
                                                name=f"pp{c}_{pi}")
            ps = proj_ps[(c, pi)]
            nc.tensor.matmul(ps[0:64, :], w_sb[:, d * DH:(d + 1) * DH], xts[d][:, 0:512],
                             start=(d == 0), stop=(d == 3), tile_position=(0, 0),
                             skip_group_check=True)
            nc.tensor.matmul(ps[64:128, :], w_sb[:, d * DH:(d + 1) * DH], xts[d][:, 512:1024],
                             start=(d == 0), stop=(d == 3), tile_position=(0, 64),
                             skip_group_check=True)
            if d == 3:
                nc.vector.tensor_copy(dst[:, CH * c:CH * c + CH], ps[:, :])
                del proj_ps[(c, pi)]
                if pi == 2:
                    del xts_pend[c]

        def queue_prep(c):
            for pi in range(3):
                for d in range(4):
                    side_q.append(lambda pi=pi, d=d: task_proj_d(c, pi, d))
            for tt in range(4 * c, 4 * c + 4):
                side_q.append(lambda tt=tt: emit_transpose_tt(tt))

        def queue_outproj(c, outTn):
            for u in range(8):
                side_q.append(lambda u=u: emit_outproj_unit(c, outTn, u))

        # ---- flat pipelined loop over all (c, jb) steps ----
        prev = None              # step whose AV is pending
        pso_cur = None           # accumulator of prev's chunk
        for c in range(NCH):
            njb = 4 * (c + 1)
            for jb in range(njb):
                cur = scores_exp(c, jb)
                if c == 0 or (c == 1 and jb < 4):
                    # dummy-matmul fillers: keep the PE array active through
                    # the DMA-bound early phase so the HAM clock gate stays
                    # open (target tile is transient and never read)
                    fp = pv_pool.tile([128, 256], f32, tag="pv", name="fill")
                    for i in range(3):
                        nc.tensor.matmul(fp[:], wu_sb[:, 128:256], wu_sb[:, 0:256],
                                         start=(i == 0), stop=(i == 2),
                                         skip_group_check=True)
                if c == 0 and jb == 0:
                    emit_proj_part(0, 2)       # v of chunk 0, overlapped with exp(0,0)
                    emit_transposes(0)
                if c == 0 and jb == 1:
                    emit_proj_qk(1)
                if c == 0 and jb == 2:
                    emit_proj_part(1, 2)
                # paced side work after scores so the next exp is not delayed
                if c >= 1 and side_q:
                    k = -(-len(side_q) // (njb - jb))      # ceil
                    for _ in range(min(k, len(side_q))):
                        side_q.popleft()()
                if jb == 3 and c + 2 < NCH and (c + 2) not in xts_pend:
                    emit_xt(c + 2)
                if prev is not None:
                    emit_av(prev, pso_cur)
                    if prev[1] == 4 * (prev[0] + 1) - 1:       # prev closed its chunk
                        outTn = emit_epilogue_a(prev[0], pso_cur)
                        queue_outproj(prev[0], outTn)
                        if prev[0] + 2 < NCH:
                            queue_prep(prev[0] + 2)
                        pso_cur = None
                if pso_cur is None:
                    pso_cur = av_pool.tile([65, 1024], f32, tag="av")
                prev = cur
            # transposes for chunk 1 come after chunk 0's short loop
            if c == 0:
                emit_transposes(1)
        emit_av(prev, pso_cur)
        outTn = emit_epilogue_a(prev[0], pso_cur)
        while side_q:                  # stragglers (chunk-6 output units)
            side_q.popleft()()
        # tail output projection: PSUM evacuations alternate between DVE and
        # the (by now idle) scalar engine.
        for u in range(8):
            emit_outproj_unit(NCH - 1, outTn, u, tail=True)
    return nc


def make_host_constants(NB: int):
    """Masks for the 4 diagonal j-block offsets and the stacked identity."""
    jj = np.arange(JB)[:, None]
    ii = np.arange(CH)[None, :]
    masks = np.zeros((128, 4096), np.float32)            # SBUF layout: mask t at cols 1024t
    for t in range(4):
        m = (ii >= jj + JB * t).astype(np.float32)       # [128, 512]
        masks[:, 1024 * t:1024 * (t + 1)] = np.concatenate([m, m], axis=1)
    identup = np.concatenate([np.eye(DH, dtype=np.float32)] * 2, axis=0)  # [128, 64]
    return (masks.astype(ml_dtypes.bfloat16), identup.astype(ml_dtypes.bfloat16))


_CACHE = {}


def _get_compiled(NB: int):
    key = ("nc", NB)
    if key not in _CACHE:
        import concourse.bacc as bacc
        nc = bacc.Bacc("TRN2", debug=False, num_devices=N_CORES)
        build_attention_kernel(nc, NB)
        nc.compile()
        _CACHE[key] = nc
    return _CACHE[key]


def make_in_maps(x, Wq, Wkv, Wo, NB: int):
    bf = ml_dtypes.bfloat16
    NB = x.shape[1]
    nb_total = x.shape[0] * NB
    xT = x.reshape(nb_total, DIM).T            # [512, B*NB], batch-major cols
    xT = xT.reshape(DIM, 2, NB // CH, CH).transpose(0, 2, 1, 3).reshape(DIM, nb_total)
    xT = np.ascontiguousarray(xT).astype(bf)   # chunk-paired: col = 1024c + 512b + i
    masks, identup = make_host_constants(NB)
    in_maps = []

    def wpack(w):        # [512, 64] -> SBUF layout [128, 256] (d-tile on free dim)
        return np.ascontiguousarray(
            w.reshape(4, 128, DH).transpose(1, 0, 2).reshape(128, 4 * DH)).astype(bf)

    for h in range(N_CORES):
        s = slice(DH * h, DH * (h + 1))
        in_maps.append({
            "xT": xT,
            "wq": wpack(Wq[:, s] / 64.0),
            "wk": wpack(Wkv[:, DH * h:DH * (h + 1)]),
            "wv": wpack(Wkv[:, DIM + DH * h:DIM + DH * (h + 1)]),
            "wo": np.ascontiguousarray(Wo[s, :]).astype(bf),
            "masks": masks,
            "identup": identup,
        })
    return in_maps


def kernel(x, Wq, Wkv, Wo, bo, _run_kwargs=None):
    from concourse.bass_utils import run_bass_kernel_spmd
    x = np.asarray(x, np.float32)
    NB = x.shape[1]
    nc = _get_compiled(NB)
    in_maps = make_in_maps(np.asarray(x), np.asarray(Wq), np.asarray(Wkv), np.asarray(Wo), NB)
    res = run_bass_kernel_spmd(nc, in_maps, core_ids=list(range(N_CORES)),
                               **(_run_kwargs or {}))
    oT = np.zeros((DIM, x.shape[0] * NB), np.float64)
    for c in range(N_CORES):
        den = res.results[c]["den"].astype(np.float64)      # [1, B*NB]
        oT += res.results[c]["oT"].astype(np.float64) / den
    # invert chunk-paired layout: col = 1024c + 512b + i  ->  [b, n, D]
    out = (oT.reshape(DIM, NB // CH, 2, CH).transpose(2, 1, 3, 0)
           .reshape(x.shape[0], NB, DIM).astype(np.float32) + np.asarray(bo, np.float32))
    if _run_kwargs is not None:
        _CACHE["last_results"] = res
    return out
